# revision 1
# baseline (speedup 1.0000x reference)
"""Trainium2 Bass kernel for a single-layer causal-attention transformer LM head.

Computation (per batch b):
    x = tok_emb[idx[b]] + pos_emb            # [T, D]
    q/k/v = x @ {Wq,Wk,Wv}.T                 # [T, D]
    attn = softmax(causal(q k^T / sqrt(D)))  # [T, T]
    x = x + 0.1 * attn @ v
    logits = x @ Wout_w.T + Wout_b           # [T, V]

Sharding over 8 cores: 2D grid (batch x vocab-half). Core c handles batch
c//2 fully (embedding, QKV, attention) and computes logits for vocab half
c%2 (16000 columns). No collectives; host concatenates the halves.

All tensors live on-chip in transposed [d, t] layout so every matmul
contraction runs over the partition axis. The entire x/q/k/v pipeline and
the logits GEMM run in bf16 (PE rate is identical to fp32r, but SBUF/DMA
traffic halves and the attention residual error is attenuated by the 0.1
scale); PSUM accumulation stays fp32. The output is written bf16 and
upconverted to f32 on the host.
"""

import numpy as np
from contextlib import ExitStack

import concourse.bass as bass
import concourse.mybir as mybir
import concourse.tile as tile
from concourse import bacc
from concourse.bass_utils import run_bass_kernel_spmd
from concourse.masks import make_identity, make_causal_mask

P = 128
F32 = mybir.dt.float32
F32R = mybir.dt.float32r
BF16 = mybir.dt.bfloat16
I32 = mybir.dt.int32

# Full-problem dims (hardcoded per spec).
B = 4
T_FULL = 2048
D_FULL = 1024
VOCAB_FULL = 32000
VH_FULL = VOCAB_FULL // 2
VCH_FULL = 500
RESIDUAL_SCALE = 0.1

_BUILD_CACHE = {}


def _build(T=T_FULL, D=D_FULL, VOCAB=VOCAB_FULL, VH=VH_FULL, VCH=VCH_FULL,
           GQ=512, use_bias=False, phases="ABC", M=1):
    """Emit the per-core Bass program. All cores run the identical NEFF;
    per-core behavior comes only from the input data.

    phases/M are profiling aids: emit only a prefix of the pipeline, or
    duplicate the whole body M times (for marginal-cost timing)."""
    NT = T // P           # t blocks
    ND = D // P           # d tiles
    NG = T // GQ          # attention query groups
    NQB = GQ // P         # tq blocks per group
    NVC = VH // VCH       # vocab chunks
    TC = min(512, T)      # projection t-chunk width
    NTC = T // TC
    VW = min(512, D)      # v projection d-chunk width
    NVW = D // VW
    scale = 1.0 / float(np.sqrt(D).astype(np.float32))

    nc = bacc.Bacc("TRN2", target_bir_lowering=False, debug=False)
    idx = nc.dram_tensor("idx", [T, 1], I32, kind="ExternalInput")
    tok = nc.dram_tensor("tok_emb", [VOCAB, D], BF16, kind="ExternalInput")
    pos = nc.dram_tensor("pos_emb", [T, D], BF16, kind="ExternalInput")
    wqT = nc.dram_tensor("wqT", [D, D], BF16, kind="ExternalInput")
    wkT = nc.dram_tensor("wkT", [D, D], BF16, kind="ExternalInput")
    wvT = nc.dram_tensor("wvT", [D, D], BF16, kind="ExternalInput")
    woutT = nc.dram_tensor("woutT", [D, VH], BF16, kind="ExternalInput")
    if use_bias:
        wout_b = nc.dram_tensor("wout_b", [1, VH], F32R, kind="ExternalInput")
    out = nc.dram_tensor("out", [T, VH], BF16, kind="ExternalOutput")

    with tile.TileContext(nc) as tc, ExitStack() as topmost:
        const = topmost.enter_context(tc.tile_pool(name="const", bufs=1))
        ident16 = const.tile([P, P], BF16)
        make_identity(nc, ident16[:])
        causal = const.tile([P, P], F32)
        make_causal_mask(nc, causal[:], -1e10)

        for _rep in range(M):
          with ExitStack() as top:
            xpool = top.enter_context(tc.tile_pool(name="xT", bufs=1))
            xT = [xpool.tile([P, T], BF16, name=f"xT{d}") for d in range(ND)]

            with ExitStack() as ab:
                kvpool = ab.enter_context(tc.tile_pool(name="kv", bufs=1))
                kT = [kvpool.tile([P, T], BF16, name=f"kT{e}") for e in range(ND)]
                vsb = [kvpool.tile([P, D], BF16, name=f"v{t}") for t in range(NT)]
                qsb = [kvpool.tile([P, T], BF16, name=f"q{e}") for e in range(ND)]
                keepalive_bf16 = list(kT) + list(vsb)

                # ---------------- Phase A: embed + transpose + QKV ----------------
                # Embedding chunks (4 t-blocks each) are interleaved with the
                # QKV matmuls for the previous chunk so the PE never starves
                # while gathers stream in. All QKV weights are SBUF-resident.
                with ExitStack() as pa:
                    ga = pa.enter_context(tc.tile_pool(name="ga", bufs=6))
                    pxp = pa.enter_context(tc.tile_pool(name="pxp", bufs=2, space="PSUM"))
                    for t in range(NT):
                        idx_t = ga.tile([P, 1], I32, name="idx_t", tag="idx_t")
                        nc.sync.dma_start(idx_t[:], idx[t * P:(t + 1) * P, :])
                        x_g = ga.tile([P, D], BF16, name="x_g", tag="x_g")
                        nc.gpsimd.indirect_dma_start(
                            out=x_g[:], out_offset=None,
                            in_=tok[:],
                            in_offset=bass.IndirectOffsetOnAxis(ap=idx_t[:, :1], axis=0))
                        pos_t = ga.tile([P, D], BF16, name="pos_t", tag="pos_t")
                        nc.sync.dma_start(pos_t[:], pos[t * P:(t + 1) * P, :])
                        xg_b = ga.tile([P, D], BF16, name="xg_b", tag="xg_b")
                        nc.vector.tensor_add(xg_b[:], x_g[:], pos_t[:])
                        for d in range(ND):
                            xps = pxp.tile([P, P], BF16, name="xps", tag="xps", space="PSUM")
                            nc.tensor.transpose(xps[:], xg_b[:, d * P:(d + 1) * P], ident16[:])
                            nc.vector.tensor_copy(xT[d][:, t * P:(t + 1) * P], xps[:])

                    wqk = pa.enter_context(tc.tile_pool(name="wqk", bufs=20))
                    pprj = pa.enter_context(tc.tile_pool(name="pprj", bufs=6, space="PSUM"))
                    for which, wT in (("k", wkT), ("q", wqT)):
                        for e in range(ND):
                            wt = []
                            for d in range(ND):
                                w_de = wqk.tile([P, P], BF16, name="w_de", tag="w_de")
                                nc.sync.dma_start(w_de[:], wT[d * P:(d + 1) * P, e * P:(e + 1) * P])
                                wt.append(w_de)
                            for c in range(NTC):
                                pq = pprj.tile([P, TC], F32, name="pq", tag="pprj_t", space="PSUM")
                                for d in range(ND):
                                    nc.tensor.matmul(pq[:], wt[d][:], xT[d][:, c * TC:(c + 1) * TC],
                                                     start=(d == 0), stop=(d == ND - 1))
                                if which == "k":
                                    nc.vector.tensor_copy(kT[e][:, c * TC:(c + 1) * TC], pq[:])
                                else:
                                    nc.vector.tensor_copy(qsb[e][:, c * TC:(c + 1) * TC], pq[:])

                    wvp = pa.enter_context(tc.tile_pool(name="wvp", bufs=16))
                    for dc in range(NVW):
                        wvt = []
                        for d in range(ND):
                            wv_t = wvp.tile([P, VW], BF16, name="wv_t", tag="wv_t")
                            nc.sync.dma_start(wv_t[:], wvT[d * P:(d + 1) * P, dc * VW:(dc + 1) * VW])
                            wvt.append(wv_t)
                        for t in range(NT):
                            pv = pprj.tile([P, VW], F32, name="pv", tag="pprj_t", space="PSUM")
                            for d in range(ND):
                                nc.tensor.matmul(pv[:], xT[d][:, t * P:(t + 1) * P], wvt[d][:],
                                                 start=(d == 0), stop=(d == ND - 1))
                            nc.vector.tensor_copy(vsb[t][:, dc * VW:(dc + 1) * VW], pv[:])

                # ---------------- Phase B: causal attention + residual ----------------
                with ExitStack() as pb:
                  if "B" in phases:
                    ppool = pb.enter_context(tc.tile_pool(name="ppool", bufs=3))
                    ptp = pb.enter_context(tc.tile_pool(name="ptp", bufs=min(20, 2 * (T // P))))
                    stat = pb.enter_context(tc.tile_pool(name="stat", bufs=10))
                    tmpp = pb.enter_context(tc.tile_pool(name="tmpp", bufs=4))
                    psc = pb.enter_context(tc.tile_pool(name="psc", bufs=4, space="PSUM"))
                    ptr = pb.enter_context(tc.tile_pool(name="ptr", bufs=2, space="PSUM"))
                    pav = pb.enter_context(tc.tile_pool(name="pav", bufs=2, space="PSUM"))
                    for g in range(NG):
                        ext = (g + 1) * GQ      # causal extent padded to group width
                        nkb = ext // P
                        qg = [qsb[e][:, g * GQ:(g + 1) * GQ] for e in range(ND)]
                        pts = [ptp.tile([P, GQ], BF16, name="pt_t", tag="pt_t")
                               for _ in range(nkb)]
                        for qb in range(NQB):
                            i = g * NQB + qb            # global tq block
                            # streaming maxless softmax: pre-mask scores are
                            # bounded (|q.k|/sqrt(D) << 1), so exp() without the
                            # rowmax shift is exact; each GQ-wide score chunk is
                            # exp'd and its PSUM bank freed right after its MMs.
                            p_sb = ppool.tile([P, T], BF16, name="p_sb", tag="p_sb")
                            rs_parts = []
                            for c in range(g + 1):
                                s_ps = psc.tile([P, GQ], F32, name="s_ps", tag="s_ps", space="PSUM")
                                for e in range(ND):
                                    nc.tensor.matmul(
                                        s_ps[:],
                                        qg[e][:, qb * P:(qb + 1) * P],
                                        kT[e][:, c * GQ:(c + 1) * GQ],
                                        start=(e == 0), stop=(e == ND - 1))
                                if c == g:
                                    nc.vector.tensor_add(s_ps[:, qb * P:(qb + 1) * P],
                                                         s_ps[:, qb * P:(qb + 1) * P],
                                                         causal[:])
                                    for j in range(qb + 1, NQB):
                                        nc.vector.memset(s_ps[:, j * P:(j + 1) * P], -1e10)
                                rsc = stat.tile([P, 1], F32, name="rsc", tag="rs")
                                nc.scalar.activation(p_sb[:, c * GQ:(c + 1) * GQ], s_ps[:],
                                                     mybir.ActivationFunctionType.Exp,
                                                     bias=0.0, scale=scale,
                                                     accum_out=rsc[:, :1])
                                rs_parts.append(rsc)
                            rtot = rs_parts[0]
                            for r2 in rs_parts[1:]:
                                nc.vector.tensor_add(rtot[:], rtot[:], r2[:])
                            ri = stat.tile([P, 1], F32, name="ri", tag="ri")
                            nc.vector.reciprocal(ri[:], rtot[:])
                            for c in range(g + 1):
                                nc.vector.tensor_scalar_mul(p_sb[:, c * GQ:(c + 1) * GQ],
                                                            p_sb[:, c * GQ:(c + 1) * GQ],
                                                            ri[:, :1])
                            for kb in range(nkb):
                                tps = ptr.tile([P, P], BF16, name="tps", tag="tps", space="PSUM")
                                nc.tensor.transpose(tps[:], p_sb[:, kb * P:(kb + 1) * P], ident16[:])
                                nc.vector.tensor_copy(pts[kb][:, qb * P:(qb + 1) * P], tps[:])
                        for d in range(ND):
                            pa_v = pav.tile([P, GQ], F32, name="pa_v", tag="pa_v", space="PSUM")
                            for kb in range(nkb):
                                nc.tensor.matmul(pa_v[:], vsb[kb][:, d * P:(d + 1) * P], pts[kb][:],
                                                 start=(kb == 0), stop=(kb == nkb - 1))
                            tmp = tmpp.tile([P, GQ], F32, name="tmp", tag="tmp")
                            nc.scalar.mul(tmp[:], pa_v[:], RESIDUAL_SCALE)
                            nc.vector.tensor_add(xT[d][:, g * GQ:(g + 1) * GQ],
                                                 xT[d][:, g * GQ:(g + 1) * GQ],
                                                 tmp[:])

                if "C" not in phases:
                    kap = ab.enter_context(tc.tile_pool(name="kap", bufs=2))
                    kidx = 0
                    for t_ in list(xT) + list(keepalive_bf16):
                        kc = kap.tile([P, P], BF16, name="kc", tag="kc")
                        nc.vector.tensor_copy(kc[:], t_[:, :P])
                        nc.sync.dma_start(out[0:P, kidx * P:(kidx + 1) * P], kc[:])
                        kidx += 1

            # ---------------- Phase C: logits GEMM ----------------
            with ExitStack() as pc:
              if "C" in phases:
                wop = pc.enter_context(tc.tile_pool(name="wop", bufs=24))
                ost = pc.enter_context(tc.tile_pool(name="ost", bufs=16))
                plg = pc.enter_context(tc.tile_pool(name="plg", bufs=8, space="PSUM"))
                if use_bias:
                    bsb = pc.enter_context(tc.tile_pool(name="bsb", bufs=1))
                    ones_f = bsb.tile([1, P], F32)
                    nc.vector.memset(ones_f[:], 1.0)
                    ones = bsb.tile([1, P], F32R)
                    nc.vector.tensor_copy(ones[:], ones_f[:])
                    bias_sb = bsb.tile([1, VH], F32R)
                    nc.sync.dma_start(bias_sb[:], wout_b[:])
                for c in range(NVC):
                    wt = []
                    for d in range(ND):
                        wo_t = wop.tile([P, VCH], BF16, name="wo_t", tag="wo_t")
                        nc.sync.dma_start(wo_t[:], woutT[d * P:(d + 1) * P, c * VCH:(c + 1) * VCH])
                        wt.append(wo_t)
                    for t in range(NT):
                        pl = plg.tile([P, VCH], F32, name="pl", tag="pl", space="PSUM")
                        for d in range(ND):
                            nc.tensor.matmul(pl[:], xT[d][:, t * P:(t + 1) * P], wt[d][:],
                                             start=(d == 0),
                                             stop=(d == ND - 1 and not use_bias))
                        if use_bias:
                            nc.tensor.matmul(pl[:], ones[:1, :P],
                                             bias_sb[:1, c * VCH:(c + 1) * VCH],
                                             start=False, stop=True)
                        ob = ost.tile([P, VCH], BF16, name="ob", tag="ob")
                        if t % 2 == 0:
                            nc.vector.tensor_copy(ob[:], pl[:])
                        else:
                            nc.scalar.copy(ob[:], pl[:])
                        nc.sync.dma_start(out[t * P:(t + 1) * P, c * VCH:(c + 1) * VCH], ob[:])


    nc.compile()
    return nc


def get_program(use_bias=False, **dims):
    key = (use_bias, tuple(sorted(dims.items())))
    if key not in _BUILD_CACHE:
        _BUILD_CACHE[key] = _build(use_bias=use_bias, **dims)
    return _BUILD_CACHE[key]


def make_in_maps(idx, tok_emb, pos_emb, Wq, Wk, Wv, Wout_w, Wout_b, n_cores=8):
    """Shard FULL inputs into per-core input maps (batch x vocab-half grid)."""
    bf16 = mybir.dt.np(BF16)
    T = idx.shape[1]
    VH = Wout_w.shape[0] // 2
    idx32 = np.ascontiguousarray(np.asarray(idx).astype(np.int32))
    tok = np.ascontiguousarray(np.asarray(tok_emb, dtype=np.float32).astype(bf16))
    posf = np.ascontiguousarray(np.asarray(pos_emb, dtype=np.float32)[:T].astype(bf16))
    WqT = np.ascontiguousarray(np.asarray(Wq, np.float32).T.astype(bf16))
    WkT = np.ascontiguousarray(np.asarray(Wk, np.float32).T.astype(bf16))
    WvT = np.ascontiguousarray(np.asarray(Wv, np.float32).T.astype(bf16))
    WoT = np.asarray(Wout_w, np.float32).T.astype(bf16)   # [D, V]
    bias = np.asarray(Wout_b, np.float32)
    use_bias = bool(np.any(bias))
    in_maps = []
    for c in range(n_cores):
        b, h = divmod(c, 2)
        m = {
            "idx": np.ascontiguousarray(idx32[b].reshape(T, 1)),
            "tok_emb": tok,
            "pos_emb": posf,
            "wqT": WqT,
            "wkT": WkT,
            "wvT": WvT,
            "woutT": np.ascontiguousarray(WoT[:, h * VH:(h + 1) * VH]),
        }
        if use_bias:
            m["wout_b"] = np.ascontiguousarray(bias[h * VH:(h + 1) * VH]).reshape(1, VH)
        in_maps.append(m)
    return in_maps, use_bias


def kernel(idx, tok_emb, pos_emb, Wq, Wk, Wv, Wout_w, Wout_b, _run_kwargs=None):
    """Full-input entry point: shards across 8 NeuronCores, runs, regathers."""
    in_maps, use_bias = make_in_maps(idx, tok_emb, pos_emb, Wq, Wk, Wv,
                                     Wout_w, Wout_b)
    nc = get_program(use_bias=use_bias)
    kw = dict(_run_kwargs or {})
    res = run_bass_kernel_spmd(nc, in_maps, core_ids=list(range(8)), **kw)
    outs = [np.asarray(r["out"]).astype(np.float32) for r in res.results]
    full = [np.concatenate([outs[2 * b], outs[2 * b + 1]], axis=1)
            for b in range(idx.shape[0])]
    result = np.stack(full).astype(np.float32)
    if _run_kwargs is not None:
        return result, res
    return result

